# revision 1
# baseline (speedup 1.0000x reference)
"""Distributed 3-layer GCN surrogate model on 8 Trainium2 NeuronCores.

Strategy (per the node-partitioned data-parallel scheme):
  - nodes are sharded across the 8 cores (12500 dst nodes each); edges are
    colocated with their destination shard and sorted by destination.
  - the norm factorizes: out_d = dinv[d] * sum_e (dinv[src_e] * (h @ W)[src_e]),
    so each layer keeps a full replicated table  hws_l = dinv * (h_{l-1} @ W_l)
    built distributed + AllGather.
  - per-edge gathers run through the Q7 dma_gather engine (4 SWDGE queues in
    parallel); segment sums per 128-edge chunk are computed on the PE as
    one-hot Sel^T @ gathered matmuls accumulating 4 chunks per PSUM bank;
    partial sums are scattered to per-destination aggregation tables with
    dma_scatter_add (each (dst, src-bucket) written exactly once -> race free).
  - the source-node axis is split in 4 buckets of 25000 rows so gather indices
    fit int16; each bucket has its own zero-initialized aggregation table and
    the post-pass sums the 4 tables, applies dinv/bias/tanh, and produces the
    next layer's table slice (transform via PE transpose + matmul).
  - final global max-pool: per-core running max -> [128, 64] output; host
    reduces over cores/partitions and applies the tiny final linear layer.
"""

import numpy as np

P = 128
SELW = 32          # one-hot Sel width: max segments per 128-edge chunk
GCALL = 32         # chunks per dma_gather call (4096 edges)
BANKC = 512        # fp32 columns per PSUM bank / staging tile
NQ = 4             # SWDGE queues
FS = [16, 32, 64]  # aggregated feature width per layer (W1/W2/W3 out dims)


# ----------------------------------------------------------------- host plan

def _wrap16(lin, dtype=np.int16):
    """Q7 index layout: idx i at [i%16, i//16], replicated to all 8 core pairs."""
    n = len(lin)
    t = np.zeros((P, n // 16), dtype)
    idx = np.arange(n)
    for k in range(8):
        t[16 * k + idx % 16, idx // 16] = lin
    return t


def build_plan(x, W1, b1, W2, b2, W3, b3, edge_index, n_cores=8):
    n = x.shape[0]
    nloc = n // n_cores
    bsz = n // 4                     # src bucket size (int16-addressable)
    nt = (nloc + P - 1) // P         # node tiles per core
    nlocp = nt * P
    tail = nloc - (nt - 1) * P       # valid rows in the last tile
    sink = nlocp                     # scatter sink row
    aggrows = nlocp + P

    src = np.concatenate([edge_index[0], np.arange(n, dtype=np.int64)]).astype(np.int64)
    dst = np.concatenate([edge_index[1], np.arange(n, dtype=np.int64)]).astype(np.int64)
    deg = np.bincount(dst, minlength=n).astype(np.float64)
    dinv = (1.0 / np.sqrt(deg)).astype(np.float32)

    core_of = dst // nloc
    per = []   # per (core, bucket): dict with srcvals/segids/pids arrays
    nchunk_max = 0
    for c in range(n_cores):
        mc = core_of == c
        s_c, d_c = src[mc], dst[mc] - c * nloc
        row = []
        for r in range(4):
            mr = (s_c // bsz) == r
            s_r = (s_c[mr] - r * bsz).astype(np.int32)
            d_r = d_c[mr].astype(np.int32)
            order = np.argsort(d_r, kind="stable")
            s_r, d_r = s_r[order], d_r[order]
            cnts = np.bincount(d_r, minlength=nloc)
            dpres = np.nonzero(cnts)[0]
            cpres = cnts[dpres]
            assert cpres.max(initial=0) <= P, "segment exceeds one chunk"
            # greedy chunk packing: <=128 edges, <=SELW segments per chunk
            seg_chunk = np.empty(len(dpres), np.int32)
            seg_slot = np.empty(len(dpres), np.int32)
            seg_off = np.empty(len(dpres), np.int32)
            ck = fill = nseg = 0
            for i, cnt in enumerate(cpres):
                if fill + cnt > P or nseg == SELW:
                    ck += 1
                    fill = nseg = 0
                seg_chunk[i] = ck
                seg_slot[i] = nseg
                seg_off[i] = fill
                fill += cnt
                nseg += 1
            nchunk = ck + 1 if len(dpres) else 0
            srcvals = np.zeros((nchunk, P), np.int16)
            segids = np.full((nchunk, P), 63.0, np.float32)
            pids = np.full((nchunk, SELW), sink, np.int16)
            estart = np.concatenate([[0], np.cumsum(cpres)[:-1]])
            ech = np.repeat(seg_chunk, cpres)
            epos = np.repeat(seg_off, cpres) + (np.arange(len(s_r)) - np.repeat(estart, cpres))
            srcvals[ech, epos] = s_r
            segids[ech, epos] = np.repeat(seg_slot, cpres).astype(np.float32)
            pids[seg_chunk, seg_slot] = dpres
            row.append((srcvals, segids, pids))
            nchunk_max = max(nchunk_max, nchunk)
        per.append(row)

    NCHUNK = ((nchunk_max + GCALL - 1) // GCALL) * GCALL
    ncalls = 4 * (NCHUNK // GCALL)

    meta = dict(n=n, n_cores=n_cores, nloc=nloc, bsz=bsz, nt=nt, nlocp=nlocp,
                tail=tail, sink=sink, aggrows=aggrows, NCHUNK=NCHUNK, ncalls=ncalls)

    # per-layer scatter staging geometry
    meta["cps"] = [4 * (BANKC // F) for F in FS]             # chunks per staging
    meta["srows"] = [P * (BANKC // F) for F in FS]           # scatter rows per staging
    meta["nstage_b"] = [(NCHUNK + cps - 1) // cps for cps in meta["cps"]]

    ins = []
    for c in range(n_cores):
        d = {}
        xs = np.ascontiguousarray(x[c * nloc:(c + 1) * nloc].T)          # [128, nloc]
        d["xT"] = xs.astype(np.float32)
        dv = np.zeros(nlocp, np.float32)
        dv[:nloc] = dinv[c * nloc:(c + 1) * nloc]
        d["dinv_pt"] = np.ascontiguousarray(dv.reshape(nt, P).T)
        ng4 = (nt + 3) // 4
        dg = np.zeros((ng4, P, 4), np.float32)
        for t in range(nt):
            dg[t // 4, :, t % 4] = dv[t * P:(t + 1) * P]
        d["dinv_g"] = dg
        d["W1p"] = W1.astype(np.float32)                                  # [128,16]
        W2p = np.zeros((64, 32), np.float32); W2p[:16] = W2
        W3p = np.zeros((64, 64), np.float32); W3p[:32] = W3
        d["W2p"], d["W3p"] = W2p, W3p
        for li, (b, F) in enumerate(zip((b1, b2, b3), FS)):
            br = np.zeros((P, 64), np.float32); br[:, :F] = b[None, :]
            d[f"brep{li}"] = br
        d["iota32"] = np.tile(np.arange(SELW, dtype=np.float32), (P, 1))
        d["ident"] = np.eye(P, dtype=np.float32)

        gidx = np.zeros((ncalls, P, GCALL * P // 16), np.int16)
        segsel = np.zeros((ncalls, P, GCALL), np.float32)
        ci = 0
        for r in range(4):
            srcvals, segids, pids = per[c][r]
            nch = srcvals.shape[0]
            sv = np.zeros((NCHUNK, P), np.int16); sv[:nch] = srcvals
            sg = np.full((NCHUNK, P), 63.0, np.float32); sg[:nch] = segids
            pd = np.full((NCHUNK, SELW), sink, np.int16); pd[:nch] = pids
            per[c][r] = pd  # keep only pids for scatter phase
            for k in range(NCHUNK // GCALL):
                blk = sv[k * GCALL:(k + 1) * GCALL]        # [32, 128]
                lin = blk[(np.arange(GCALL * P) // P), (np.arange(GCALL * P) % P)]
                gidx[ci] = _wrap16(lin)
                segsel[ci] = sg[k * GCALL:(k + 1) * GCALL].T
                ci += 1
        d["gidx"] = gidx
        d["segsel"] = segsel

        for li, F in enumerate(FS):
            cps, srows, nst_b = meta["cps"][li], meta["srows"][li], meta["nstage_b"][li]
            sp = np.zeros((4 * nst_b, P, srows // 16), np.int16)
            si = 0
            for r in range(4):
                pd = per[c][r]
                for st in range(nst_b):
                    i = np.arange(srows)
                    cc = (i % P) // SELW + 4 * (i // P)
                    s = i % SELW
                    k = st * cps + cc
                    pid = np.where(k < NCHUNK, pd[np.minimum(k, NCHUNK - 1), s], sink)
                    sp[si] = _wrap16(pid.astype(np.int16))
                    si += 1
            d[f"spid{li}"] = sp
        for li in range(3):
            for r in range(4):
                d[f"aggz{li}_{r}"] = np.zeros((aggrows, 64), np.float32)
        ins.append(d)
    return ins, meta, dinv


# --------------------------------------------------------------- bass program

def _dma_gather_raw(nc, out_ap, in_ap, idxs_ap, num_idxs, elem_size, elem_step, queue_num):
    """bass dma_gather with the elem%256B assert relaxed (ucode only requires
    the table stride to be a multiple of 256B)."""
    import concourse.mybir as mybir
    g = nc.gpsimd
    g._assert_queue_num(queue_num)
    stride_bytes = elem_step * mybir.dt.size(in_ap.dtype)
    assert stride_bytes % 256 == 0
    _in_ap = g.lower_ap_dma(in_ap, for_custom_bir_dma=True)
    return g.add_instruction(
        mybir.InstDMAGatherAnt(
            name=g.bass.get_next_instruction_name(),
            ins=[*_in_ap, g.lower_ap(idxs_ap), g.lower_val_access(g.to_reg(num_idxs))],
            outs=[g.lower_ap(out_ap)],
            transpose=False, num_idxs=num_idxs, elem_size=elem_size,
            stride_bytes_256=stride_bytes // 256, gen_mode=0,
            single_packet=False, queue_num=queue_num,
            sbuf_tokens_per_rank=0, sbuf_free_dim_per_rank=0,
            sbuf_free_dim_pad_per_rank=0, sbuf_byte_offset=0,
        ))


def build_program(meta):
    import concourse.bass as bass
    import concourse.bacc as bacc
    import concourse.tile as tile
    import concourse.mybir as mybir
    f32, i16 = mybir.dt.float32, mybir.dt.int16

    n, n_cores = meta["n"], meta["n_cores"]
    nloc, bsz, nt, tail = meta["nloc"], meta["bsz"], meta["nt"], meta["tail"]
    aggrows, NCHUNK, ncalls = meta["aggrows"], meta["NCHUNK"], meta["ncalls"]

    nc = bacc.Bacc("TRN2", target_bir_lowering=False, debug=False,
                   num_devices=n_cores, num_swdge_queues=NQ)

    t_in = {}
    t_in["xT"] = nc.dram_tensor("xT", [P, nloc], f32, kind="ExternalInput")
    t_in["dinv_pt"] = nc.dram_tensor("dinv_pt", [P, nt], f32, kind="ExternalInput")
    ng4 = (nt + 3) // 4
    t_in["dinv_g"] = nc.dram_tensor("dinv_g", [ng4, P, 4], f32, kind="ExternalInput")
    t_in["W1p"] = nc.dram_tensor("W1p", [P, 16], f32, kind="ExternalInput")
    t_in["W2p"] = nc.dram_tensor("W2p", [64, 32], f32, kind="ExternalInput")
    t_in["W3p"] = nc.dram_tensor("W3p", [64, 64], f32, kind="ExternalInput")
    for li in range(3):
        t_in[f"brep{li}"] = nc.dram_tensor(f"brep{li}", [P, 64], f32, kind="ExternalInput")
    t_in["iota32"] = nc.dram_tensor("iota32", [P, SELW], f32, kind="ExternalInput")
    t_in["ident"] = nc.dram_tensor("ident", [P, P], f32, kind="ExternalInput")
    t_in["gidx"] = nc.dram_tensor("gidx", [ncalls, P, GCALL * P // 16], i16, kind="ExternalInput")
    t_in["segsel"] = nc.dram_tensor("segsel", [ncalls, P, GCALL], f32, kind="ExternalInput")
    for li in range(3):
        t_in[f"spid{li}"] = nc.dram_tensor(
            f"spid{li}", [4 * meta["nstage_b"][li], P, meta["srows"][li] // 16], i16,
            kind="ExternalInput")
        for r in range(4):
            t_in[f"aggz{li}_{r}"] = nc.dram_tensor(
                f"aggz{li}_{r}", [aggrows, 64], f32, kind="ExternalInput")
    pooled = nc.dram_tensor("pooled", [P, 64], f32, kind="ExternalOutput")

    tables = [nc.dram_tensor(f"hws{li}", [n, 64], f32, addr_space="Shared")
              for li in range(3)]
    bounces = [nc.dram_tensor(f"bounce{li}", [nloc, 64], f32) for li in range(3)]

    with tile.TileContext(nc) as tc:
        with (
            tc.tile_pool(name="const", bufs=1) as cpool,
            tc.tile_pool(name="gt", bufs=6) as gt_pool,
            tc.tile_pool(name="gi", bufs=6) as gi_pool,
            tc.tile_pool(name="seg", bufs=4) as seg_pool,
            tc.tile_pool(name="sel", bufs=4) as sel_pool,
            tc.tile_pool(name="sp", bufs=4) as sp_pool,
            tc.tile_pool(name="stage", bufs=4) as st_pool,
            tc.tile_pool(name="post", bufs=3) as post_pool,
            tc.tile_pool(name="hw", bufs=3) as hw_pool,
            tc.tile_pool(name="acc", bufs=1) as acc_pool,
            tc.tile_pool(name="pagg", bufs=3, space="PSUM") as pagg,
            tc.tile_pool(name="pmm", bufs=2, space="PSUM") as pmm,
        ):
            # ---- constants
            xT = cpool.tile([P, nloc], f32)
            nc.sync.dma_start(xT[:], t_in["xT"].ap())
            W1t = cpool.tile([P, 16], f32)
            nc.sync.dma_start(W1t[:], t_in["W1p"].ap())
            W2t = cpool.tile([64, 32], f32)
            nc.sync.dma_start(W2t[:], t_in["W2p"].ap())
            W3t = cpool.tile([64, 64], f32)
            nc.sync.dma_start(W3t[:], t_in["W3p"].ap())
            breps = []
            for li in range(3):
                bt = cpool.tile([P, 64], f32, tag=f"brep{li}")
                nc.sync.dma_start(bt[:], t_in[f"brep{li}"].ap())
                breps.append(bt)
            iota = cpool.tile([P, SELW], f32)
            nc.sync.dma_start(iota[:], t_in["iota32"].ap())
            ident = cpool.tile([P, P], f32)
            nc.sync.dma_start(ident[:], t_in["ident"].ap())
            dinv_cols = cpool.tile([P, nt], f32)   # dinv col per node tile
            nc.sync.dma_start(dinv_cols[:], t_in["dinv_pt"].ap())

            qrr = [0]

            def next_q():
                q = qrr[0] % NQ
                qrr[0] += 1
                return q

            # ---- phase A: hws0 = dinv * (x @ W1), distributed + AllGather
            for t in range(nt):
                m = P if t < nt - 1 else tail
                ps = pmm.tile([P, 64], f32, space="PSUM", tag="mm")
                nc.tensor.matmul(ps[:m, :16], xT[:, t * P:t * P + m], W1t[:],
                                 start=True, stop=True)
                hw = hw_pool.tile([P, 64], f32)
                nc.vector.tensor_scalar_mul(hw[:m, :16], ps[:m, :16],
                                            dinv_cols[:m, t:t + 1])
                nc.sync.dma_start(bounces[0].ap()[t * P:t * P + m, :16], hw[:m, :16])
            nc.gpsimd.collective_compute(
                "AllGather", mybir.AluOpType.bypass,
                replica_groups=[list(range(n_cores))],
                ins=[bounces[0].ap()], outs=[tables[0].ap()])

            # ---- layers
            acc = acc_pool.tile([P, 64], f32)
            for li in range(3):
                F = FS[li]
                cps, srows, nst_b = meta["cps"][li], meta["srows"][li], meta["nstage_b"][li]
                table = tables[li]
                # gather + Sel matmul + scatter partials, per src bucket
                cpg = cps // GCALL              # gather calls per staging
                for r in range(4):
                    agg_t = t_in[f"aggz{li}_{r}"]
                    gt = sel = None
                    for st in range(nst_b):
                        ps_st = pagg.tile([P, BANKC], f32, space="PSUM", tag="agg")
                        for kc in range(cpg):
                            k = st * cpg + kc
                            if k < NCHUNK // GCALL:
                                ci = r * (NCHUNK // GCALL) + k
                                it = gi_pool.tile([P, GCALL * P // 16], i16)
                                nc.sync.dma_start(it[:], t_in["gidx"].ap()[ci])
                                gt = gt_pool.tile([P, GCALL * F], f32)
                                _dma_gather_raw(
                                    nc, gt[:].rearrange("p (g f) -> p g f", f=F),
                                    table.ap()[r * bsz:(r + 1) * bsz, :F],
                                    it[:], GCALL * P, F, 64, next_q())
                                sg = seg_pool.tile([P, GCALL], f32)
                                nc.sync.dma_start(sg[:], t_in["segsel"].ap()[ci])
                                sel = sel_pool.tile([P, GCALL * SELW], f32)
                                for w in range(GCALL * SELW // BANKC):
                                    cw = BANKC // SELW
                                    nc.vector.tensor_tensor(
                                        out=sel[:, w * BANKC:(w + 1) * BANKC],
                                        in0=sg[:, w * cw:(w + 1) * cw]
                                            .rearrange("p (c o) -> p c o", o=1)
                                            .to_broadcast([P, cw, SELW]),
                                        in1=iota[:].rearrange("p (o s) -> p o s", o=1)
                                            .to_broadcast([P, cw, SELW]),
                                        op=mybir.AluOpType.is_equal)
                            for c in range(GCALL):
                                cc = kc * GCALL + c            # chunk in staging
                                pg, slot = cc % 4, cc // 4
                                nc.tensor.matmul(
                                    ps_st[pg * 32:(pg + 1) * 32, slot * F:(slot + 1) * F],
                                    sel[:, c * SELW:(c + 1) * SELW],
                                    gt[:, c * F:(c + 1) * F],
                                    start=True, stop=True,
                                    tile_position=(0, pg * 32))
                        st_tile = st_pool.tile([P, BANKC], f32)
                        nc.scalar.copy(out=st_tile[:], in_=ps_st[:])
                        spt = sp_pool.tile([P, srows // 16], i16)
                        nc.sync.dma_start(spt[:], t_in[f"spid{li}"].ap()[r * nst_b + st])
                        nc.gpsimd.dma_scatter_add(
                            out_ap=agg_t.ap()[:, :F],
                            in_ap=st_tile[:].rearrange("p (g f) -> p g f", f=F),
                            idxs_ap=spt[:], num_idxs=srows, num_idxs_reg=srows,
                            elem_size=F, elem_step=64,
                            single_packet=False, queue_num=next_q())

                # ---- post-pass over node tiles (groups of 4)
                for g in range(ng4):
                    t0 = g * 4
                    gt_n = min(4, nt - t0)
                    w = gt_n * 64
                    rows = t0 * P
                    a = []
                    for r in range(4):
                        at = post_pool.tile([P, 256], f32, tag=f"aggrd{r}")
                        nc.sync.dma_start(
                            at[:, :w],
                            t_in[f"aggz{li}_{r}"].ap()[rows:rows + gt_n * P, :]
                            .rearrange("(j p) f -> p j f", p=P))
                        a.append(at)
                    s01 = post_pool.tile([P, 256], f32, tag="s01")
                    nc.vector.tensor_tensor(out=s01[:, :w], in0=a[0][:, :w],
                                            in1=a[1][:, :w], op=mybir.AluOpType.add)
                    s23 = post_pool.tile([P, 256], f32, tag="s23")
                    nc.vector.tensor_tensor(out=s23[:, :w], in0=a[2][:, :w],
                                            in1=a[3][:, :w], op=mybir.AluOpType.add)
                    v = post_pool.tile([P, 256], f32, tag="v")
                    nc.vector.tensor_tensor(out=v[:, :w], in0=s01[:, :w],
                                            in1=s23[:, :w], op=mybir.AluOpType.add)
                    dg = post_pool.tile([P, 4], f32, tag="dg")
                    nc.sync.dma_start(dg[:], t_in["dinv_g"].ap()[g])
                    nc.vector.tensor_tensor(
                        out=v[:, :w], in0=v[:, :w],
                        in1=dg[:, :gt_n].rearrange("p (j o) -> p j o", o=1)
                            .to_broadcast([P, gt_n, 64]),
                        op=mybir.AluOpType.mult)
                    nc.vector.tensor_tensor(
                        out=v[:, :w], in0=v[:, :w],
                        in1=breps[li][:].rearrange("p (o f) -> p o f", o=1)
                            .to_broadcast([P, gt_n, 64]),
                        op=mybir.AluOpType.add)
                    h = post_pool.tile([P, 256], f32, tag="h")
                    nc.scalar.activation(h[:, :w], v[:, :w],
                                         mybir.ActivationFunctionType.Tanh)
                    for j in range(gt_n):
                        t = t0 + j
                        m = P if t < nt - 1 else tail
                        if li == 2:
                            hs = h[:m, j * 64:(j + 1) * 64]
                            if t == 0:
                                nc.vector.tensor_copy(out=acc[:m, :], in_=hs)
                            else:
                                nc.vector.tensor_tensor(out=acc[:m, :], in0=acc[:m, :],
                                                        in1=hs, op=mybir.AluOpType.max)
                        else:
                            psT = pmm.tile([P, P], f32, space="PSUM", tag="mmT")
                            nc.tensor.transpose(psT[:64, :], h[:, j * 64:(j + 1) * 64],
                                                ident[:])
                            hT = post_pool.tile([64, P], f32, tag="hT")
                            nc.scalar.copy(out=hT[:], in_=psT[:64, :])
                            Wn = W2t if li == 0 else W3t
                            Fn = FS[li + 1]
                            ps2 = pmm.tile([P, 64], f32, space="PSUM", tag="mm")
                            nc.tensor.matmul(ps2[:m, :Fn], hT[:, :m], Wn[:],
                                             start=True, stop=True)
                            hw = hw_pool.tile([P, 64], f32)
                            nc.vector.tensor_scalar_mul(hw[:m, :Fn], ps2[:m, :Fn],
                                                        dinv_cols[:m, t:t + 1])
                            nc.sync.dma_start(
                                bounces[li + 1].ap()[t * P:t * P + m, :Fn],
                                hw[:m, :Fn])
                if li < 2:
                    nc.gpsimd.collective_compute(
                        "AllGather", mybir.AluOpType.bypass,
                        replica_groups=[list(range(n_cores))],
                        ins=[bounces[li + 1].ap()], outs=[tables[li + 1].ap()])
            nc.sync.dma_start(pooled.ap(), acc[:])
    nc.compile()
    return nc


_CACHE = {}


def kernel(x, W1, b1, W2, b2, W3, b3, Wl, bl, edge_index):
    x = np.asarray(x); edge_index = np.asarray(edge_index)
    ins, meta, _ = build_plan(x, np.asarray(W1), np.asarray(b1), np.asarray(W2),
                              np.asarray(b2), np.asarray(W3), np.asarray(b3),
                              edge_index, n_cores=8)
    key = (x.shape, edge_index.shape, meta["NCHUNK"])
    if key not in _CACHE:
        _CACHE[key] = build_program(meta)
    nc = _CACHE[key]
    from concourse.bass_utils import run_bass_kernel_spmd
    res = run_bass_kernel_spmd(nc, ins, core_ids=list(range(8)))
    pool = np.stack([res.results[c]["pooled"] for c in range(8)])  # [8,128,64]
    pooled = pool.max(axis=(0, 1))[:64].astype(np.float32)          # [64]
    out = pooled[None, :] @ np.asarray(Wl, np.float32) + np.asarray(bl, np.float32)
    return out.astype(np.float32)



# revision 8
# speedup vs baseline: 2.1894x; 2.1894x over previous
"""Distributed 3-layer GCN on 8 Trainium2 NeuronCores — v2.

Structure (node-partitioned, dst-colocated edges):
  - nodes sharded 8 ways (12500 dsts/core); edges live with their dst core,
    sorted by (src-bucket, dst-window, dst). 4 src-buckets of 25000 keep
    gather indices in int16.
  - per layer l, a replicated DRAM table [n, 128] bf16 holds
    dinv_s * (h @ W_l) rows (only first F_l cols used; 256B row stride
    satisfies the SWDGE gather stride rule). Built distributed + AllGather.
  - per (bucket, 512-dst window): one dma_gather call fetches the group's
    edges' source rows (rank-split into NCOLG columns of 128 edges, padded
    with idx=-1 -> cheap dummy descriptors). A host-built one-hot-ish sel
    strip (slot match x dinv_d, built on-chip from 2 small DVE ops) turns
    each column into a matmul: psum[:F, a_k:a_k+SELW_k] += gathered.T @ sel.
    All 4 buckets accumulate into the same PSUM window; no scatter-add, no
    aggregation tables, no post-pass.
  - window close: ACT does tanh(psum + b) -> h^T tile [F, 512] bf16; the
    transposed tile is directly the lhsT for the next-layer projection
    (h @ W_{l+1}), scaled by dinv and DMA'd to the bounce for AllGather.
  - layer 3: running max over windows -> [64, 512] per core; host reduces
    and applies the final linear layer.
  - SPMD: program constants (NCOLG, sel grid a_k/SELW_k) are maxima over
    all cores computed from the actual graph; per-core variability lives in
    the gather-index / sel-meta data only.
"""

import numpy as np
import ml_dtypes

BF16 = ml_dtypes.bfloat16

P = 128
NB = 4             # src buckets (int16 gather idx)
W = 512            # dsts per PSUM window
CHUNK = 128        # edges per column (= matmul contraction)
FS = [16, 32, 64]  # per-layer aggregated feature width
TW = 128           # table row width in bf16 (256B stride)
NQ = 4             # SWDGE queues


def _wrap16(lin, dtype=np.int16):
    """Q7 index layout: idx i at [16k + i%16, i//16] for the 8 core pairs."""
    n = len(lin)
    t = np.zeros((P, n // 16), dtype)
    idx = np.arange(n)
    for k in range(8):
        t[16 * k + idx % 16, idx // 16] = lin
    return t


# ----------------------------------------------------------------- host plan

def build_plan(x, W1, b1, W2, b2, W3, b3, edge_index, n_cores=8):
    n = x.shape[0]
    nloc = n // n_cores
    bsz = n // NB
    nwin = (nloc + W - 1) // W
    nt = (nloc + P - 1) // P
    ng = NB * nwin

    src = np.concatenate([edge_index[0], np.arange(n, dtype=np.int64)])
    dst = np.concatenate([edge_index[1], np.arange(n, dtype=np.int64)])
    deg = np.bincount(dst, minlength=n).astype(np.float64)
    dinv = (1.0 / np.sqrt(np.maximum(deg, 1.0))).astype(np.float32)

    core_of = dst // nloc
    percore = []
    ncolg = 0
    cnt_max = np.zeros(ng, np.int64)
    for c in range(n_cores):
        m = core_of == c
        s, d = src[m], dst[m] - c * nloc
        r = s // bsz
        srel = (s - r * bsz).astype(np.int32)
        w = d // W
        slot = (d - w * W).astype(np.int32)
        g = (r * nwin + w).astype(np.int32)
        order = np.lexsort((slot, g))
        srel, slot, g, dg = srel[order], slot[order], g[order], d[order]
        cnt = np.bincount(g, minlength=ng)
        cnt_max = np.maximum(cnt_max, cnt)
        start = np.concatenate([[0], np.cumsum(cnt)[:-1]])
        rank = np.arange(len(g)) - start[g]
        col = rank // CHUNK
        ncolg = max(ncolg, int(col.max(initial=0)) + 1)
        percore.append((srel, slot, g, dg + c * nloc, col, rank, cnt))
    ncols_g = ((cnt_max + CHUNK - 1) // CHUNK).astype(np.int64)

    # global sel grid: per column index k, [a_k, a_k+SELW_k) covers every
    # core/group's column-k slot range (program constants)
    lo = np.full(ncolg, W, np.int64)
    hi = np.full(ncolg, -1, np.int64)
    for srel, slot, g, dgl, col, rank, cnt in percore:
        np.minimum.at(lo, col, slot)
        np.maximum.at(hi, col, slot)
    lo = np.minimum(lo, hi)  # empty cols -> degenerate
    selw = (hi - lo + 1).clip(1)
    sw = int(selw.max())
    assert sw <= 256, sw  # bf16-exact slot values

    meta = dict(n=n, n_cores=n_cores, nloc=nloc, bsz=bsz, nwin=nwin, nt=nt,
                ng=ng, ncolg=ncolg, sw=sw, a=lo.tolist(), selw=selw.tolist(),
                ncols_g=ncols_g.tolist())

    ins = []
    for c in range(n_cores):
        srel, slot, g, dgl, col, rank, cnt = percore[c]
        # within-column sort by src for gather locality
        cg = g * ncolg + col
        o2 = np.lexsort((srel, cg))
        srel, slot, g, dgl, col = srel[o2], slot[o2], g[o2], dgl[o2], col[o2]
        pos = np.arange(len(g)) - np.concatenate(
            [[0], np.cumsum(np.bincount(cg[o2], minlength=ng * ncolg))[:-1]])[cg[o2]]

        gl = np.zeros((ng, ncolg * CHUNK), np.int32)
        gl[g, col * CHUNK + pos] = srel
        slm = np.full((ng, P, 2 * ncolg), 0, np.float32)
        slm[:, :, :ncolg] = -1.0
        slm[g, pos, col] = (slot - lo[col]).astype(np.float32)
        slm[g, pos, ncolg + col] = dinv[dgl]

        gidx = np.zeros((ng, P, ncolg * CHUNK // 16), np.int16)
        for gi in range(ng):
            gidx[gi] = _wrap16(gl[gi])

        d = {}
        d["gidx"] = gidx
        d["selmeta"] = slm.astype(BF16)
        xs = x[c * nloc:(c + 1) * nloc].astype(np.float32)
        d["xTbf"] = np.ascontiguousarray(xs.T).astype(BF16)
        dv = np.zeros(nt * P, np.float32)
        dv[:nloc] = dinv[c * nloc:(c + 1) * nloc]
        d["dinv_pt"] = np.ascontiguousarray(dv.reshape(nt, P).T)
        d["W1bf"] = np.asarray(W1, np.float32).astype(BF16)
        d["W2bf"] = np.asarray(W2, np.float32).astype(BF16)
        d["W3bf"] = np.asarray(W3, np.float32).astype(BF16)
        d["b1t"] = np.asarray(b1, np.float32).reshape(-1, 1)
        d["b2t"] = np.asarray(b2, np.float32).reshape(-1, 1)
        d["b3t"] = np.asarray(b3, np.float32).reshape(-1, 1)
        d["iotaSW"] = np.tile(np.arange(sw, dtype=np.float32),
                              (P, 1)).astype(BF16)
        ins.append(d)
    return ins, meta, dinv


# --------------------------------------------------------------- bass program

def _dma_gather_raw(nc, out_ap, in_ap, idxs_ap, num_idxs, elem_size, elem_step,
                    queue_num):
    """bass dma_gather with the elem%256B assert relaxed (ucode only needs
    the table stride to be a multiple of 256B)."""
    import concourse.mybir as mybir
    g = nc.gpsimd
    g._assert_queue_num(queue_num)
    stride_bytes = elem_step * mybir.dt.size(in_ap.dtype)
    assert stride_bytes % 256 == 0
    _in_ap = g.lower_ap_dma(in_ap, for_custom_bir_dma=True)
    return g.add_instruction(
        mybir.InstDMAGatherAnt(
            name=g.bass.get_next_instruction_name(),
            ins=[*_in_ap, g.lower_ap(idxs_ap), g.lower_val_access(g.to_reg(num_idxs))],
            outs=[g.lower_ap(out_ap)],
            transpose=False, num_idxs=num_idxs, elem_size=elem_size,
            stride_bytes_256=stride_bytes // 256, gen_mode=0,
            single_packet=False, queue_num=queue_num,
            sbuf_tokens_per_rank=0, sbuf_free_dim_per_rank=0,
            sbuf_free_dim_pad_per_rank=0, sbuf_byte_offset=0,
        ))


def build_program(meta):
    import concourse.bacc as bacc
    import concourse.tile as tile
    import concourse.mybir as mybir
    f32, i16, bf16 = mybir.dt.float32, mybir.dt.int16, mybir.dt.bfloat16

    n, n_cores, nloc = meta["n"], meta["n_cores"], meta["nloc"]
    bsz, nwin, nt, ng = meta["bsz"], meta["nwin"], meta["nt"], meta["ng"]
    ncolg, sw = meta["ncolg"], meta["sw"]
    A, SELW, NCOLS_G = meta["a"], meta["selw"], meta["ncols_g"]
    NIDX = ncolg * CHUNK

    nc = bacc.Bacc("TRN2", target_bir_lowering=False, debug=False,
                   num_devices=n_cores, num_swdge_queues=NQ)

    t_in = {}
    t_in["xTbf"] = nc.dram_tensor("xTbf", [P, nloc], bf16, kind="ExternalInput")
    t_in["dinv_pt"] = nc.dram_tensor("dinv_pt", [P, nt], f32, kind="ExternalInput")
    t_in["W1bf"] = nc.dram_tensor("W1bf", [P, 16], bf16, kind="ExternalInput")
    t_in["W2bf"] = nc.dram_tensor("W2bf", [16, 32], bf16, kind="ExternalInput")
    t_in["W3bf"] = nc.dram_tensor("W3bf", [32, 64], bf16, kind="ExternalInput")
    t_in["b1t"] = nc.dram_tensor("b1t", [16, 1], f32, kind="ExternalInput")
    t_in["b2t"] = nc.dram_tensor("b2t", [32, 1], f32, kind="ExternalInput")
    t_in["b3t"] = nc.dram_tensor("b3t", [64, 1], f32, kind="ExternalInput")
    t_in["iotaSW"] = nc.dram_tensor("iotaSW", [P, sw], bf16, kind="ExternalInput")
    t_in["gidx"] = nc.dram_tensor("gidx", [ng, P, NIDX // 16], i16,
                                  kind="ExternalInput")
    t_in["selmeta"] = nc.dram_tensor("selmeta", [ng, P, 2 * ncolg], bf16,
                                     kind="ExternalInput")
    t_out = nc.dram_tensor("pool", [64, W], f32, kind="ExternalOutput")

    tables = [nc.dram_tensor(f"tab{li}", [n, TW], bf16, addr_space="Shared")
              for li in range(3)]
    bounces = [nc.dram_tensor(f"bnc{li}", [nloc, TW], bf16) for li in range(3)]

    with tile.TileContext(nc) as tc:
        with (
            tc.tile_pool(name="const", bufs=1) as cpool,
            tc.tile_pool(name="gi", bufs=3) as gi_pool,
            tc.tile_pool(name="gt", bufs=3) as gt_pool,
            tc.tile_pool(name="sm", bufs=3) as sm_pool,
            tc.tile_pool(name="sel", bufs=3) as sel_pool,
            tc.tile_pool(name="hT", bufs=2) as h_pool,
            tc.tile_pool(name="stage", bufs=2) as st_pool,
            tc.tile_pool(name="pagg", bufs=2, space="PSUM") as pagg,
            tc.tile_pool(name="pmm", bufs=2, space="PSUM") as pmm,
        ):
            xT = cpool.tile([P, nloc], bf16)
            nc.sync.dma_start(xT[:], t_in["xTbf"].ap())
            dinv_pt = cpool.tile([P, nt], f32)
            nc.sync.dma_start(dinv_pt[:], t_in["dinv_pt"].ap())
            W1t = cpool.tile([P, 16], bf16)
            nc.sync.dma_start(W1t[:], t_in["W1bf"].ap())
            W2t = cpool.tile([16, 32], bf16)
            nc.sync.dma_start(W2t[:], t_in["W2bf"].ap())
            W3t = cpool.tile([32, 64], bf16)
            nc.sync.dma_start(W3t[:], t_in["W3bf"].ap())
            bts = []
            for li, fdim in enumerate(FS):
                bt = cpool.tile([fdim, 1], f32, tag=f"b{li}")
                nc.sync.dma_start(bt[:], t_in[f"b{li+1}t"].ap())
                bts.append(bt)
            iota = cpool.tile([P, sw], bf16)
            nc.sync.dma_start(iota[:], t_in["iotaSW"].ap())
            acc = cpool.tile([64, W], f32, tag="acc")
            nc.vector.memset(acc[:], -2.0)

            qrr = [0]

            def next_q():
                q = qrr[0] % NQ
                qrr[0] += 1
                return q

            def project(w, lhsT_fn, Wn, Fn, bounce):
                """h(-tile)@Wn -> *dinv -> bf16 -> bounce rows for window w."""
                wlen = min(W, nloc - w * W)
                njt = (wlen + P - 1) // P
                stage = st_pool.tile([P, njt * Fn], bf16, tag="stage")
                for j in range(njt):
                    t = w * (W // P) + j
                    m = min(P, wlen - j * P)
                    ps2 = pmm.tile([P, 64], f32, space="PSUM", tag="proj")
                    nc.tensor.matmul(ps2[:m, :Fn], lhsT_fn(j, m), Wn[:],
                                     start=True, stop=True)
                    nc.vector.tensor_scalar_mul(stage[:m, j * Fn:(j + 1) * Fn],
                                                ps2[:m, :Fn],
                                                dinv_pt[:m, t:t + 1])
                if wlen == W:
                    nc.sync.dma_start(
                        bounce.ap()[w * W:(w + 1) * W, :Fn]
                        .rearrange("(j p) f -> p j f", p=P),
                        stage[:].rearrange("p (j f) -> p j f", f=Fn))
                else:
                    for j in range(njt):
                        m = min(P, wlen - j * P)
                        nc.sync.dma_start(
                            bounce.ap()[w * W + j * P:w * W + j * P + m, :Fn],
                            stage[:m, j * Fn:(j + 1) * Fn])

            # ---- phase 0: table0 rows = dinv * (x @ W1)
            for w in range(nwin):
                project(w, lambda j, m, _w=w: xT[:, (_w * (W // P) + j) * P:
                                                 (_w * (W // P) + j) * P + m],
                        W1t, 16, bounces[0])
            nc.gpsimd.collective_compute(
                "AllGather", mybir.AluOpType.bypass,
                replica_groups=[list(range(n_cores))],
                ins=[bounces[0].ap()], outs=[tables[0].ap()])

            # ---- layers
            for li in range(3):
                F = FS[li]
                table = tables[li]
                for w in range(nwin):
                    wlen = min(W, nloc - w * W)
                    psw = pagg.tile([64, W], f32, space="PSUM", tag="agg")
                    nc.vector.memset(psw[:], 0.0)
                    for r in range(NB):
                        g = r * nwin + w
                        ncg = NCOLS_G[g]
                        if ncg == 0:
                            continue
                        nig = ncg * CHUNK
                        it = gi_pool.tile([P, nig // 16], i16, tag=f"gi{r}")
                        nc.sync.dma_start(it[:], t_in["gidx"].ap()[g][:, :nig // 16])
                        gt = gt_pool.tile([P, ncg * F], bf16, tag=f"gt{r}")
                        _dma_gather_raw(
                            nc, gt[:].rearrange("p (c f) -> p c f", f=F),
                            table.ap()[r * bsz:(r + 1) * bsz, :F],
                            it[:], nig, F, TW, next_q())
                        sm = sm_pool.tile([P, 2 * ncolg], bf16, tag=f"sm{r}")
                        nc.sync.dma_start(sm[:], t_in["selmeta"].ap()[g])
                        sel = sel_pool.tile([P, ncg * sw], bf16, tag=f"sel{r}")
                        sel3 = sel[:].rearrange("p (c s) -> p c s", s=sw)
                        nc.vector.tensor_tensor(
                            out=sel3,
                            in0=sm[:, :ncg].rearrange("p (c o) -> p c o", o=1)
                                .to_broadcast([P, ncg, sw]),
                            in1=iota[:].rearrange("p (o s) -> p o s", o=1)
                                .to_broadcast([P, ncg, sw]),
                            op=mybir.AluOpType.is_equal)
                        nc.vector.tensor_tensor(
                            out=sel3, in0=sel3,
                            in1=sm[:, ncolg:ncolg + ncg]
                                .rearrange("p (c o) -> p c o", o=1)
                                .to_broadcast([P, ncg, sw]),
                            op=mybir.AluOpType.mult)
                        for k in range(ncg):
                            a, sl = A[k], SELW[k]
                            nc.tensor.matmul(
                                psw[:F, a:a + sl],
                                gt[:, k * F:(k + 1) * F],
                                sel[:, k * sw:k * sw + sl],
                                start=False, stop=True, skip_group_check=True)
                    if li < 2:
                        hT = h_pool.tile([64, W], bf16, tag="hT")
                        nc.scalar.activation(hT[:F, :wlen], psw[:F, :wlen],
                                             mybir.ActivationFunctionType.Tanh,
                                             bias=bts[li][:, :1])
                        Wn = W2t if li == 0 else W3t
                        Fn = FS[li + 1]
                        project(w, lambda j, m, _hT=hT, _F=F:
                                _hT[:_F, j * P:j * P + m],
                                Wn, Fn, bounces[li + 1])
                    else:
                        hTf = h_pool.tile([64, W], f32, tag="hTf")
                        nc.scalar.activation(hTf[:64, :wlen], psw[:64, :wlen],
                                             mybir.ActivationFunctionType.Tanh,
                                             bias=bts[2][:, :1])
                        nc.vector.tensor_tensor(out=acc[:, :wlen],
                                                in0=acc[:, :wlen],
                                                in1=hTf[:, :wlen],
                                                op=mybir.AluOpType.max)
                if li < 2:
                    nc.gpsimd.collective_compute(
                        "AllGather", mybir.AluOpType.bypass,
                        replica_groups=[list(range(n_cores))],
                        ins=[bounces[li + 1].ap()], outs=[tables[li + 1].ap()])
            nc.sync.dma_start(t_out.ap(), acc[:])
    nc.compile()
    return nc


_CACHE = {}


def kernel(x, W1, b1, W2, b2, W3, b3, Wl, bl, edge_index):
    x = np.asarray(x)
    edge_index = np.asarray(edge_index)
    ins, meta, _ = build_plan(x, W1, b1, W2, b2, W3, b3, edge_index, n_cores=8)
    key = (x.shape, edge_index.shape, meta["ncolg"], meta["sw"],
           tuple(meta["a"]), tuple(meta["selw"]), tuple(meta["ncols_g"]))
    if key not in _CACHE:
        _CACHE[key] = build_program(meta)
    nc = _CACHE[key]
    from concourse.bass_utils import run_bass_kernel_spmd
    res = run_bass_kernel_spmd(nc, ins, core_ids=list(range(8)))
    pool = np.stack([np.asarray(res.results[c]["pool"], np.float32)
                     for c in range(8)])            # [8, 64, 512]
    pooled = pool.max(axis=(0, 2))                  # [64]
    out = pooled[None, :] @ np.asarray(Wl, np.float32) + np.asarray(bl, np.float32)
    return out.astype(np.float32)


# revision 15
# speedup vs baseline: 2.8734x; 1.3124x over previous
"""Distributed 3-layer GCN on 8 Trainium2 NeuronCores — v2.

Structure (node-partitioned, dst-colocated edges):
  - nodes sharded 8 ways (12500 dsts/core); edges live with their dst core,
    sorted by (src-bucket, dst-window, dst). 4 src-buckets of 25000 keep
    gather indices in int16.
  - per layer l, a replicated DRAM table [n, 128] bf16 holds
    dinv_s * (h @ W_l) rows (only first F_l cols used; 256B row stride
    satisfies the SWDGE gather stride rule). Built distributed + AllGather.
  - per (bucket, 512-dst window): one dma_gather call fetches the group's
    edges' source rows (rank-split into NCOLG columns of 128 edges, padded
    with idx=-1 -> cheap dummy descriptors). A host-built one-hot-ish sel
    strip (slot match x dinv_d, built on-chip from 2 small DVE ops) turns
    each column into a matmul: psum[:F, a_k:a_k+SELW_k] += gathered.T @ sel.
    All 4 buckets accumulate into the same PSUM window; no scatter-add, no
    aggregation tables, no post-pass.
  - window close: ACT does tanh(psum + b) -> h^T tile [F, 512] bf16; the
    transposed tile is directly the lhsT for the next-layer projection
    (h @ W_{l+1}), scaled by dinv and DMA'd to the bounce for AllGather.
  - layer 3: running max over windows -> [64, 512] per core; host reduces
    and applies the final linear layer.
  - SPMD: program constants (NCOLG, sel grid a_k/SELW_k) are maxima over
    all cores computed from the actual graph; per-core variability lives in
    the gather-index / sel-meta data only.
"""

import numpy as np
import ml_dtypes

BF16 = ml_dtypes.bfloat16

P = 128
NB = 4             # src buckets (int16 gather idx)
W = 512            # dsts per PSUM window
CHUNK = 128        # edges per column (= matmul contraction)
FS = [16, 32, 64]  # per-layer aggregated feature width
TW = 128           # table row width in bf16 (256B stride)
NQ = 4             # SWDGE queues


def _wrap16(lin, dtype=np.int16):
    """Q7 index layout: idx i at [16k + i%16, i//16] for the 8 core pairs."""
    n = len(lin)
    t = np.zeros((P, n // 16), dtype)
    idx = np.arange(n)
    for k in range(8):
        t[16 * k + idx % 16, idx // 16] = lin
    return t


# ----------------------------------------------------------------- host plan

def build_plan(x, W1, b1, W2, b2, W3, b3, edge_index, n_cores=8):
    n = x.shape[0]
    nloc = n // n_cores
    bsz = n // NB
    nwin = (nloc + W - 1) // W
    nt = (nloc + P - 1) // P
    ng = NB * nwin

    src = np.concatenate([edge_index[0], np.arange(n, dtype=np.int64)])
    dst = np.concatenate([edge_index[1], np.arange(n, dtype=np.int64)])
    deg = np.bincount(dst, minlength=n).astype(np.float64)
    dinv = (1.0 / np.sqrt(np.maximum(deg, 1.0))).astype(np.float32)

    core_of = dst // nloc
    percore = []
    cnt_max = np.zeros(ng, np.int64)
    for c in range(n_cores):
        m = core_of == c
        s, d = src[m], dst[m] - c * nloc
        r = s // bsz
        srel = (s - r * bsz).astype(np.int64)
        w = d // W
        slot = (d - w * W).astype(np.int64)
        g = (r * nwin + w).astype(np.int64)
        order = np.lexsort((slot, g))
        srel, slot, g, dg = srel[order], slot[order], g[order], d[order]
        cnt = np.bincount(g, minlength=ng)
        cnt_max = np.maximum(cnt_max, cnt)
        percore.append((srel, slot, g, dg + c * nloc, cnt))
    ncols_g = ((cnt_max + CHUNK - 1) // CHUNK).astype(np.int64)
    ncolg = int(ncols_g.max())

    # quantile-split columns: col of rank i in group g = i*ncg // cnt
    # (balanced <=128/col, slot ranges tight across cores). Sel grid
    # [a,a+selw) per (g,k) = min/max over cores (program constants).
    lo = np.full((ng, ncolg), W, np.int64)
    hi = np.full((ng, ncolg), -1, np.int64)
    pc2 = []
    for srel, slot, g, dgl, cnt in percore:
        start = np.concatenate([[0], np.cumsum(cnt)[:-1]])
        rank = np.arange(len(g)) - start[g]
        col = rank * ncols_g[g] // np.maximum(cnt[g], 1)
        np.minimum.at(lo, (g, col), slot)
        np.maximum.at(hi, (g, col), slot)
        pc2.append((srel, slot, g, dgl, col))
    lo = np.minimum(lo, np.maximum(hi, 0))
    selw = (hi - lo + 1).clip(1)
    sw = int(selw.max())
    assert sw <= 256, sw  # bf16-exact slot values

    meta = dict(n=n, n_cores=n_cores, nloc=nloc, bsz=bsz, nwin=nwin, nt=nt,
                ng=ng, ncolg=ncolg, sw=sw, a=lo.tolist(), selw=selw.tolist(),
                ncols_g=ncols_g.tolist())

    ins = []
    for c in range(n_cores):
        srel, slot, g, dgl, col = pc2[c]
        # within-column sort by src for gather locality
        cg = g * ncolg + col
        o2 = np.lexsort((srel, cg))
        srel, slot, g, dgl, col = srel[o2], slot[o2], g[o2], dgl[o2], col[o2]
        pos = np.arange(len(g)) - np.concatenate(
            [[0], np.cumsum(np.bincount(cg[o2], minlength=ng * ncolg))[:-1]])[cg[o2]]

        gl = np.zeros((ng, ncolg * CHUNK), np.int32)
        gl[g, col * CHUNK + pos] = srel
        selall = np.zeros((ng, P, ncolg * sw), BF16)
        selall[g, pos, col * sw + slot - lo[g, col]] = 1.0

        gidx = np.zeros((ng, P, ncolg * CHUNK // 16), np.int16)
        for gi in range(ng):
            gidx[gi] = _wrap16(gl[gi])

        dvr = np.zeros((nwin, 64, W), np.float32)
        dl = dinv[c * nloc:(c + 1) * nloc]
        for w in range(nwin):
            wl = min(W, nloc - w * W)
            dvr[w, :, :wl] = dl[w * W:w * W + wl][None, :]

        d = {}
        d["gidx"] = gidx
        d["selall"] = selall
        d["dinvrep"] = dvr
        xs = x[c * nloc:(c + 1) * nloc].astype(np.float32)
        d["xTbf"] = np.ascontiguousarray(xs.T).astype(BF16)
        dv = np.zeros(nt * P, np.float32)
        dv[:nloc] = dinv[c * nloc:(c + 1) * nloc]
        d["dinv_pt"] = np.ascontiguousarray(dv.reshape(nt, P).T)
        d["W1bf"] = np.asarray(W1, np.float32).astype(BF16)
        d["W2bf"] = np.asarray(W2, np.float32).astype(BF16)
        d["W3bf"] = np.asarray(W3, np.float32).astype(BF16)
        d["b1t"] = np.asarray(b1, np.float32).reshape(-1, 1)
        d["b2t"] = np.asarray(b2, np.float32).reshape(-1, 1)
        d["b3t"] = np.asarray(b3, np.float32).reshape(-1, 1)
        ins.append(d)
    return ins, meta, dinv


# --------------------------------------------------------------- bass program

def _dma_gather_raw(nc, out_ap, in_ap, idxs_ap, num_idxs, elem_size, elem_step,
                    queue_num):
    """bass dma_gather with the elem%256B assert relaxed (ucode only needs
    the table stride to be a multiple of 256B)."""
    import concourse.mybir as mybir
    g = nc.gpsimd
    g._assert_queue_num(queue_num)
    stride_bytes = elem_step * mybir.dt.size(in_ap.dtype)
    assert stride_bytes % 256 == 0
    _in_ap = g.lower_ap_dma(in_ap, for_custom_bir_dma=True)
    return g.add_instruction(
        mybir.InstDMAGatherAnt(
            name=g.bass.get_next_instruction_name(),
            ins=[*_in_ap, g.lower_ap(idxs_ap), g.lower_val_access(g.to_reg(num_idxs))],
            outs=[g.lower_ap(out_ap)],
            transpose=False, num_idxs=num_idxs, elem_size=elem_size,
            stride_bytes_256=stride_bytes // 256, gen_mode=0,
            single_packet=False, queue_num=queue_num,
            sbuf_tokens_per_rank=0, sbuf_free_dim_per_rank=0,
            sbuf_free_dim_pad_per_rank=0, sbuf_byte_offset=0,
        ))


def build_program(meta):
    import concourse.bacc as bacc
    import concourse.tile as tile
    import concourse.mybir as mybir
    f32, i16, bf16 = mybir.dt.float32, mybir.dt.int16, mybir.dt.bfloat16

    n, n_cores, nloc = meta["n"], meta["n_cores"], meta["nloc"]
    bsz, nwin, nt, ng = meta["bsz"], meta["nwin"], meta["nt"], meta["ng"]
    ncolg, sw = meta["ncolg"], meta["sw"]
    A, SELW, NCOLS_G = meta["a"], meta["selw"], meta["ncols_g"]
    NIDX = ncolg * CHUNK

    nc = bacc.Bacc("TRN2", target_bir_lowering=False, debug=False,
                   num_devices=n_cores, num_swdge_queues=NQ)

    t_in = {}
    t_in["xTbf"] = nc.dram_tensor("xTbf", [P, nloc], bf16, kind="ExternalInput")
    t_in["dinv_pt"] = nc.dram_tensor("dinv_pt", [P, nt], f32, kind="ExternalInput")
    t_in["W1bf"] = nc.dram_tensor("W1bf", [P, 16], bf16, kind="ExternalInput")
    t_in["W2bf"] = nc.dram_tensor("W2bf", [16, 32], bf16, kind="ExternalInput")
    t_in["W3bf"] = nc.dram_tensor("W3bf", [32, 64], bf16, kind="ExternalInput")
    t_in["b1t"] = nc.dram_tensor("b1t", [16, 1], f32, kind="ExternalInput")
    t_in["b2t"] = nc.dram_tensor("b2t", [32, 1], f32, kind="ExternalInput")
    t_in["b3t"] = nc.dram_tensor("b3t", [64, 1], f32, kind="ExternalInput")
    t_in["gidx"] = nc.dram_tensor("gidx", [ng, P, NIDX // 16], i16,
                                  kind="ExternalInput")
    t_in["selall"] = nc.dram_tensor("selall", [ng, P, ncolg * sw], bf16,
                                    kind="ExternalInput")
    t_in["dinvrep"] = nc.dram_tensor("dinvrep", [nwin, 64, W], f32,
                                     kind="ExternalInput")
    t_out = nc.dram_tensor("pool", [64, W], f32, kind="ExternalOutput")

    tables = [nc.dram_tensor(f"tab{li}", [n, TW], bf16, addr_space="Shared")
              for li in range(3)]
    bounces = [nc.dram_tensor(f"bnc{li}", [nloc, TW], bf16) for li in range(3)]

    with tile.TileContext(nc) as tc:
        with (
            tc.tile_pool(name="const", bufs=1) as cpool,
            tc.tile_pool(name="gi", bufs=3) as gi_pool,
            tc.tile_pool(name="gt", bufs=3) as gt_pool,
            tc.tile_pool(name="sel", bufs=3) as sel_pool,
            tc.tile_pool(name="dv", bufs=2) as dv_pool,
            tc.tile_pool(name="hT", bufs=2) as h_pool,
            tc.tile_pool(name="stage", bufs=2) as st_pool,
            tc.tile_pool(name="pagg", bufs=2, space="PSUM") as pagg,
            tc.tile_pool(name="pmm", bufs=2, space="PSUM") as pmm,
        ):
            xT = cpool.tile([P, nloc], bf16)
            nc.sync.dma_start(xT[:], t_in["xTbf"].ap())
            dinv_pt = cpool.tile([P, nt], f32)
            nc.sync.dma_start(dinv_pt[:], t_in["dinv_pt"].ap())
            W1t = cpool.tile([P, 16], bf16)
            nc.sync.dma_start(W1t[:], t_in["W1bf"].ap())
            W2t = cpool.tile([16, 32], bf16)
            nc.sync.dma_start(W2t[:], t_in["W2bf"].ap())
            W3t = cpool.tile([32, 64], bf16)
            nc.sync.dma_start(W3t[:], t_in["W3bf"].ap())
            bts = []
            for li, fdim in enumerate(FS):
                bt = cpool.tile([fdim, 1], f32, tag=f"b{li}")
                nc.sync.dma_start(bt[:], t_in[f"b{li+1}t"].ap())
                bts.append(bt)
            acc = cpool.tile([64, W], f32, tag="acc")
            nc.vector.memset(acc[:], -2.0)

            qrr = [0]

            def next_q():
                q = qrr[0] % NQ
                qrr[0] += 1
                return q

            def project(w, lhsT_fn, Wn, Fn, bounce):
                """h(-tile)@Wn -> *dinv -> bf16 -> bounce rows for window w."""
                wlen = min(W, nloc - w * W)
                njt = (wlen + P - 1) // P
                stage = st_pool.tile([P, njt * Fn], bf16, tag="stage")
                for j in range(njt):
                    t = w * (W // P) + j
                    m = min(P, wlen - j * P)
                    ps2 = pmm.tile([P, 64], f32, space="PSUM", tag="proj")
                    nc.tensor.matmul(ps2[:m, :Fn], lhsT_fn(j, m), Wn[:],
                                     start=True, stop=True)
                    nc.vector.tensor_scalar_mul(stage[:m, j * Fn:(j + 1) * Fn],
                                                ps2[:m, :Fn],
                                                dinv_pt[:m, t:t + 1])
                if wlen == W:
                    nc.sync.dma_start(
                        bounce.ap()[w * W:(w + 1) * W, :Fn]
                        .rearrange("(j p) f -> p j f", p=P),
                        stage[:].rearrange("p (j f) -> p j f", f=Fn))
                else:
                    for j in range(njt):
                        m = min(P, wlen - j * P)
                        nc.sync.dma_start(
                            bounce.ap()[w * W + j * P:w * W + j * P + m, :Fn],
                            stage[:m, j * Fn:(j + 1) * Fn])

            # ---- phase 0: table0 rows = dinv * (x @ W1)
            for w in range(nwin):
                project(w, lambda j, m, _w=w: xT[:, (_w * (W // P) + j) * P:
                                                 (_w * (W // P) + j) * P + m],
                        W1t, 16, bounces[0])
            nc.gpsimd.collective_compute(
                "AllGather", mybir.AluOpType.bypass,
                replica_groups=[list(range(n_cores))],
                ins=[bounces[0].ap()], outs=[tables[0].ap()])

            # ---- layers
            for li in range(3):
                F = FS[li]
                table = tables[li]
                for w in range(nwin):
                    wlen = min(W, nloc - w * W)
                    psw = pagg.tile([64, W], f32, space="PSUM", tag="agg")
                    nc.vector.memset(psw[:], 0.0)
                    for r in range(NB):
                        g = r * nwin + w
                        ncg = NCOLS_G[g]
                        if ncg == 0:
                            continue
                        nig = ncg * CHUNK
                        it = gi_pool.tile([P, nig // 16], i16, tag=f"gi{r}")
                        nc.sync.dma_start(it[:], t_in["gidx"].ap()[g][:, :nig // 16])
                        gt = gt_pool.tile([P, ncg * F], bf16, tag=f"gt{r}")
                        _dma_gather_raw(
                            nc, gt[:].rearrange("p (c f) -> p c f", f=F),
                            table.ap()[r * bsz:(r + 1) * bsz, :F],
                            it[:], nig, F, TW, next_q())
                        sel = sel_pool.tile([P, ncg * sw], bf16, tag=f"sel{r}")
                        nc.sync.dma_start(sel[:],
                                          t_in["selall"].ap()[g][:, :ncg * sw])
                        for k in range(ncg):
                            a, sl = A[g][k], SELW[g][k]
                            nc.tensor.matmul(
                                psw[:F, a:a + sl],
                                gt[:, k * F:(k + 1) * F],
                                sel[:, k * sw:k * sw + sl],
                                start=False, stop=True, skip_group_check=True)
                    dv = dv_pool.tile([64, W], f32, tag="dv")
                    nc.sync.dma_start(dv[:], t_in["dinvrep"].ap()[w])
                    nc.vector.tensor_tensor(out=psw[:F, :wlen],
                                            in0=psw[:F, :wlen],
                                            in1=dv[:F, :wlen],
                                            op=mybir.AluOpType.mult)
                    if li < 2:
                        hT = h_pool.tile([64, W], bf16, tag="hT")
                        nc.scalar.activation(hT[:F, :wlen], psw[:F, :wlen],
                                             mybir.ActivationFunctionType.Tanh,
                                             bias=bts[li][:, :1])
                        Wn = W2t if li == 0 else W3t
                        Fn = FS[li + 1]
                        project(w, lambda j, m, _hT=hT, _F=F:
                                _hT[:_F, j * P:j * P + m],
                                Wn, Fn, bounces[li + 1])
                    else:
                        hTf = h_pool.tile([64, W], f32, tag="hTf")
                        nc.scalar.activation(hTf[:64, :wlen], psw[:64, :wlen],
                                             mybir.ActivationFunctionType.Tanh,
                                             bias=bts[2][:, :1])
                        nc.vector.tensor_tensor(out=acc[:, :wlen],
                                                in0=acc[:, :wlen],
                                                in1=hTf[:, :wlen],
                                                op=mybir.AluOpType.max)
                if li < 2:
                    nc.gpsimd.collective_compute(
                        "AllGather", mybir.AluOpType.bypass,
                        replica_groups=[list(range(n_cores))],
                        ins=[bounces[li + 1].ap()], outs=[tables[li + 1].ap()])
            nc.sync.dma_start(t_out.ap(), acc[:])
    nc.compile()
    return nc


_CACHE = {}


def kernel(x, W1, b1, W2, b2, W3, b3, Wl, bl, edge_index):
    x = np.asarray(x)
    edge_index = np.asarray(edge_index)
    ins, meta, _ = build_plan(x, W1, b1, W2, b2, W3, b3, edge_index, n_cores=8)
    key = (x.shape, edge_index.shape, meta["ncolg"], meta["sw"],
           tuple(map(tuple, meta["a"])), tuple(map(tuple, meta["selw"])),
           tuple(meta["ncols_g"]))
    if key not in _CACHE:
        _CACHE[key] = build_program(meta)
    nc = _CACHE[key]
    from concourse.bass_utils import run_bass_kernel_spmd
    res = run_bass_kernel_spmd(nc, ins, core_ids=list(range(8)))
    pool = np.stack([np.asarray(res.results[c]["pool"], np.float32)
                     for c in range(8)])            # [8, 64, 512]
    pooled = pool.max(axis=(0, 2))                  # [64]
    out = pooled[None, :] @ np.asarray(Wl, np.float32) + np.asarray(bl, np.float32)
    return out.astype(np.float32)
